# revision 13
# baseline (speedup 1.0000x reference)
"""ChameleonAttention on 8 Trainium2 NeuronCores.

Tensor-parallel over heads: each core owns 4 of the 32 heads.
  - Wq/Wk/Wv sharded column-wise (512 cols/core), Wo row-wise (512 rows/core)
  - per-head LayerNorm + RoPE computed on-chip, gamma/beta replicated
  - causal attention with block-skipping (only lower-triangular key tiles)
  - per-core partial output [S, HID] summed on host (the TP all-reduce)

Phase P (fp16 operands): hT and the per-projection weights are SBUF-resident
per S-half; each (t, m) unit accumulates its full K=4096 contraction in a
dedicated PSUM bank (m-outer, kk-inner: units finish staggered, so LN/RoPE
tails pipeline behind the next units' matmuls instead of bunching). LN stats
(bn_stats/bn_aggr) read PSUM directly on DVE; RoPE uses host-precomputed
512-wide fp16 coefficients (packed c1/dd/ee, one DMA per unit) with
rotate-half as two half-width shuffled-AP multiplies on DVE's 16-bit path.
q/k head tiles reach [d, s] layout via DMA XBAR transposes issued from the
(idle) scalar-engine queue. DMAs are batched into multi-tile transfers to
keep the SP sequencer's ~1us-per-DMA issue cost off the critical path.

Attention query-banks qb0/qb1 depend only on S-half 0, so they are emitted
at the half boundary to keep PE busy during the half-1 hT reload. at_t
aliases qt_t (a query bank's qt columns are dead once its scores are done).
Softmax uses exp(s*scale - 4) with no running max (LayerNormed q/k bound the
logits), denominator via an all-ones stationary matmul, division deferred
to after the P@V accumulation.

Out-proj: m-outer with all 8 PSUM banks per s-tile (stationary at-tiles
reused across the 8 Wo column panels), drains alternate ACT/DVE, one 1MB
output DMA per s-tile. Wo tiles reuse the weight pool's rotation.
"""
import math
from contextlib import ExitStack

import numpy as np

_S = 2048
_HID = 4096
_D = 128
_NC = 8
_CPW = _HID // _NC  # columns per core (512) = 4 heads
_HPC = _CPW // _D  # heads per core (4)
_ROPE_THETA = 10000.0
_EPS = 1e-5
_EXP_BIAS = -4.0

_cache = {}


def _build(S, niter=1, knobs=None):
    kb = {'ht': 8, 'wp': 16, 'cp': 3, 'xn': 2, 'tm': 3, 'st': 16,
          'up': 3, 'ot': 2, 'ps': 8, 'rp': 2}
    kb.update(knobs or {})
    import concourse.tile as tile
    from concourse import bacc, mybir

    f32 = mybir.dt.float32
    f16 = mybir.dt.float16
    mul = mybir.AluOpType.mult
    add = mybir.AluOpType.add

    NM = S // 128  # s-tiles (16)
    NQB = S // 512  # query banks (4)
    NMH = NM // 2  # s-tiles per half (8)
    SH = S // 2  # rows per half
    NK = _HID // 128  # contraction k-tiles (32)
    hd = _D // 2
    scale = 1.0 / math.sqrt(_D)

    nc = bacc.Bacc("TRN2", target_bir_lowering=False, debug=False)

    hT_d = nc.dram_tensor("hT", [_HID, S], f16, kind="ExternalInput")
    w_d = {
        t: nc.dram_tensor(f"w{t}", [_HID, _CPW], f16, kind="ExternalInput")
        for t in ("q", "k", "v")
    }
    wo_d = nc.dram_tensor("wo", [_CPW, _HID], f16, kind="ExternalInput")
    rope_d = {
        t: nc.dram_tensor(f"rope{t}", [S, 3 * _CPW], f16, kind="ExternalInput")
        for t in ("q", "k")
    }
    masks_d = nc.dram_tensor("masks", [4, 128, 512], f16, kind="ExternalInput")
    out_d = nc.dram_tensor("out", [S, _HID], f16, kind="ExternalOutput")

    for _it in range(niter):
      _p = f'i{_it}_' if niter > 1 else ''
      with tile.TileContext(nc) as tc, ExitStack() as ctx:
          persist = ctx.enter_context(tc.tile_pool(name=f"{_p}persist", bufs=1))
          ones16 = persist.tile([128, 128], f16)
          nc.vector.memset(ones16[:], 1.0)
          ebias = persist.tile([128, 1], f32)
          nc.vector.memset(ebias[:], _EXP_BIAS)
          epst = persist.tile([128, 1], f32)
          nc.vector.memset(epst[:], _EPS)

          att = ctx.enter_context(tc.tile_pool(name=f"{_p}att", bufs=1))
          qt_t = [att.tile([128, S], f16, name=f"{_p}qt{h}") for h in range(_HPC)]
          kt_t = [att.tile([128, S], f16, name=f"{_p}kt{h}") for h in range(_HPC)]
          v16 = att.tile([128, NM, 512], f16)
          at_t = qt_t  # at[h][:, qb] written after qb's scores consumed qt

          mpool = ctx.enter_context(tc.tile_pool(name=f"{_p}mp", bufs=1))
          mask_t = mpool.tile([128, 4, 512], f16)
          nc.scalar.dma_start(mask_t[:], masks_d.ap().rearrange("t p n -> p t n"))

          htp = ctx.enter_context(tc.tile_pool(name=f"{_p}htp", bufs=kb["ht"]))
          wpool = ctx.enter_context(tc.tile_pool(name=f"{_p}wp", bufs=kb["wp"]))
          cpool = ctx.enter_context(tc.tile_pool(name=f"{_p}cp", bufs=kb["cp"]))
          xnp = ctx.enter_context(tc.tile_pool(name=f"{_p}xnp", bufs=kb["xn"]))
          tmp = ctx.enter_context(tc.tile_pool(name=f"{_p}tmp", bufs=kb["tm"]))
          stp = ctx.enter_context(tc.tile_pool(name=f"{_p}stp", bufs=kb["st"]))
          upool = ctx.enter_context(tc.tile_pool(name=f"{_p}up", bufs=kb["up"]))
          rpool = ctx.enter_context(tc.tile_pool(name=f"{_p}rp", bufs=kb["rp"]))
          otp = ctx.enter_context(tc.tile_pool(name=f"{_p}otp", bufs=kb["ot"]))
          psp = ctx.enter_context(
              tc.tile_pool(name=f"{_p}psp", bufs=kb["ps"], space="PSUM"))

          def ps_tile(nm):
              return psp.tile([128, 512], f32, tag="ps", name=f"{_p}{nm}")

          def w_tile(nm):
              # shared rotation for projection weight groups AND wo panels
              return wpool.tile([128, 4, 512], f16, tag="w", name=f"{_p}{nm}")

          def _tail(t, half, m, x):
              """LN + RoPE for PSUM unit x=[128, 512] of (t, m); t in q,k."""
              dst = qt_t if t == "q" else kt_t
              gm = half * NMH + m
              cf = cpool.tile([128, 3, _HPC, _D], f16, tag="cf",
                              name=f"{_p}cf_{half}{t}{m}")
              nc.scalar.dma_start(
                  cf[:], rope_d[t][gm * 128:(gm + 1) * 128, :]
              )
              xn = xnp.tile([128, _HPC, _D], f16, tag="xn",
                            name=f"{_p}xn_{half}{t}{m}")
              mvs, rstds = [], []
              for h in range(_HPC):
                  st = stp.tile([128, 6], f32, tag="st",
                                name=f"{_p}st_{half}{t}{m}{h}")
                  mv = stp.tile([128, 2], f32, tag="mv",
                                name=f"{_p}mv_{half}{t}{m}{h}")
                  nc.vector.bn_stats(out=st[:], in_=x[:, h * _D:(h + 1) * _D])
                  nc.vector.bn_aggr(out=mv[:], in_=st[:])
                  mvs.append(mv)
              for h in range(_HPC):
                  rstd = stp.tile([128, 1], f32, tag="rs",
                                  name=f"{_p}rs_{half}{t}{m}{h}")
                  nc.scalar.activation(
                      out=rstd[:], in_=mvs[h][:, 1:2],
                      func=mybir.ActivationFunctionType.Sqrt,
                      bias=epst[:], scale=1.0,
                  )
                  rstds.append(rstd)
              for h in range(_HPC):
                  nc.vector.reciprocal(out=rstds[h][:], in_=rstds[h][:])
                  nc.vector.tensor_scalar(
                      out=xn[:, h, :], in0=x[:, h * _D:(h + 1) * _D],
                      scalar1=mvs[h][:, 0:1], scalar2=rstds[h][:],
                      op0=mybir.AluOpType.subtract, op1=mul,
                  )
              t1 = tmp.tile([128, _HPC, _D], f16, tag="t1",
                            name=f"{_p}t1_{half}{t}{m}")
              t2 = tmp.tile([128, _HPC, _D], f16, tag="t2",
                            name=f"{_p}t2_{half}{t}{m}")
              # t1 = xn*(g*cos); t2 = rot_half(xn)*(sign*g[perm]*sin)
              nc.vector.tensor_tensor(t1[:], xn[:], cf[:, 0], op=mul)
              nc.vector.tensor_tensor(
                  t2[:, :, :hd], xn[:, :, hd:], cf[:, 1, :, :hd], op=mul)
              nc.vector.tensor_tensor(
                  t2[:, :, hd:], xn[:, :, :hd], cf[:, 1, :, hd:], op=mul)
              nc.vector.tensor_tensor(t1[:], t1[:], t2[:], op=add)
              nc.vector.tensor_tensor(t1[:], t1[:], cf[:, 2], op=add)
              for h in range(_HPC):
                  nc.scalar.dma_start_transpose(
                      dst[h][:, gm * 128:(gm + 1) * 128], t1[:, h, :]
                  )

          def emit_A(qb):
              """Attention for one 512-query bank, heads in pairs so the
              softmax division's PSUM wait is already satisfied at issue."""
              nkt = 4 * qb + 4
              for hp in range(_HPC // 2):
                pair = {}
                for h in (2 * hp, 2 * hp + 1):
                  o_ps = ps_tile(f"o_{h}_{qb}")
                  d_ps = ps_tile(f"d_{h}_{qb}")
                  pair[h] = (o_ps, d_ps)
                  for kt in range(nkt):
                      s_ps = ps_tile(f"s_{h}_{qb}_{kt}")
                      nc.tensor.matmul(
                          s_ps[:],
                          kt_t[h][:, kt * 128:(kt + 1) * 128],
                          qt_t[h][:, qb * 512:(qb + 1) * 512],
                          start=True, stop=True,
                      )
                      u = upool.tile([128, 512], f16, tag="u",
                                     name=f"{_p}u_{h}_{qb}_{kt}")
                      nc.scalar.activation(
                          out=u[:], in_=s_ps[:],
                          func=mybir.ActivationFunctionType.Exp,
                          bias=ebias[:], scale=scale,
                      )
                      toff = kt - 4 * qb
                      if toff >= 0:
                          nc.vector.tensor_tensor(
                              u[:], u[:], mask_t[:, toff, :], op=mul
                          )
                      nc.tensor.matmul(
                          o_ps[:], v16[:, kt, h * _D:(h + 1) * _D], u[:],
                          start=(kt == 0), stop=(kt == nkt - 1),
                      )
                      nc.tensor.matmul(
                          d_ps[:], ones16[:], u[:],
                          start=(kt == 0), stop=(kt == nkt - 1),
                      )
                for h in (2 * hp, 2 * hp + 1):
                  o_ps, d_ps = pair[h]
                  rec = rpool.tile([128, 512], f16, tag="r",
                                   name=f"{_p}r_{h}_{qb}")
                  with nc.allow_low_precision(reason="softmax denom in f16"):
                      nc.vector.reciprocal(out=rec[:], in_=d_ps[:])
                  nc.vector.tensor_tensor(
                      at_t[h][:, qb * 512:(qb + 1) * 512], o_ps[:], rec[:],
                      op=mul,
                  )

          # ================= phase P + interleaved attention ============
          wog = []
          for half in range(2):
              htg = []
              wg = {t: [] for t in ("q", "k", "v")}
              for g4 in range(8):
                  ht = htp.tile([128, 4, SH], f16, tag="ht",
                                name=f"{_p}ht_{half}_{g4}")
                  wt = w_tile(f"wq_{half}_{g4}")
                  if g4 == 0:  # weights first: smaller, unblocks matmul 0
                      nc.sync.dma_start(
                          wt[:],
                          w_d["q"][g4 * 512:(g4 + 1) * 512, :].rearrange(
                              "(g p) n -> p g n", p=128),
                      )
                  nc.sync.dma_start(
                      ht[:],
                      hT_d[g4 * 512:(g4 + 1) * 512,
                           half * SH:(half + 1) * SH].rearrange(
                               "(g p) s -> p g s", p=128),
                  )
                  htg.append(ht)
                  if g4 != 0:
                      nc.sync.dma_start(
                          wt[:],
                          w_d["q"][g4 * 512:(g4 + 1) * 512, :].rearrange(
                              "(g p) n -> p g n", p=128),
                      )
                  wg["q"].append(wt)
              for t in ("k", "v"):
                  for g4 in range(8):
                      wt = w_tile(f"w{t}_{half}_{g4}")
                      nc.sync.dma_start(
                          wt[:],
                          w_d[t][g4 * 512:(g4 + 1) * 512, :].rearrange(
                              "(g p) n -> p g n", p=128),
                      )
                      wg[t].append(wt)

              def hts(kk):
                  return htg[kk // 4][:, kk % 4, :]

              for t in ("q", "k", "v"):
                  for m in range(NMH):
                      ps = ps_tile(f"p_{half}_{t}_{m}")
                      for kk in range(NK):
                          nc.tensor.matmul(
                              ps[:],
                              hts(kk)[:, m * 128:(m + 1) * 128],
                              wg[t][kk // 4][:, kk % 4, :],
                              start=(kk == 0),
                              stop=(kk == NK - 1),
                          )
                      if t == "v":
                          gm = half * NMH + m
                          if m % 2 == 0:
                              nc.vector.tensor_copy(v16[:, gm, :], ps[:])
                          else:
                              nc.scalar.activation(
                                  out=v16[:, gm, :], in_=ps[:],
                                  func=mybir.ActivationFunctionType.Copy,
                              )
                      else:
                          _tail(t, half, m, ps[:])
              if half == 0:
                  emit_A(0)
                  emit_A(1)
              else:
                  # wo loads after half-1's weight DMAs: they alias half-1 wk
                  # slots (released end of its k section), landing in time
                  # for phase O right after half-1 P
                  for k4 in range(_HPC):
                      for j in range(2):
                          wo_t = w_tile(f"wo_{k4}_{j}")
                          nc.sync.dma_start(
                              wo_t[:],
                              wo_d[k4 * 128:(k4 + 1) * 128,
                                   j * 2048:(j + 1) * 2048].rearrange(
                                       "p (g n) -> p g n", n=512),
                          )
                          wog.append(wo_t)

          def emit_O(m):
              psx = [ps_tile(f"x_{m}_{n}") for n in range(8)]
              for k4 in range(_HPC):
                  for n in range(8):
                      nc.tensor.matmul(
                          psx[n][:],
                          at_t[k4][:, m * 128:(m + 1) * 128],
                          wog[2 * k4 + n // 4][:, n % 4, :],
                          start=(k4 == 0), stop=(k4 == _HPC - 1),
                      )
              for j in range(4):
                  ot = otp.tile([128, 2, 512], f16, tag="ot",
                                name=f"{_p}ot_{m}_{j}")
                  for nn in range(2):
                      n = 2 * j + nn
                      if n % 2 == 0:
                          nc.scalar.activation(
                              out=ot[:, nn, :], in_=psx[n][:],
                              func=mybir.ActivationFunctionType.Copy,
                          )
                      else:
                          nc.vector.tensor_copy(ot[:, nn, :], psx[n][:])
                  nc.sync.dma_start(
                      out_d[m * 128:(m + 1) * 128, j * 1024:(j + 1) * 1024],
                      ot[:])

          # O for the first S-half fills the half-1 tail window before A2
          for m in range(NMH):
              emit_O(m)
          emit_A(2)
          for m in range(NMH, NMH + 4):
              emit_O(m)
          emit_A(3)
          for m in range(NMH + 4, NM):
              emit_O(m)

    nc.compile()
    return nc


def _host_prep(hidden_states, position_ids, Wq, Wk, Wv, Wo, qn_w, qn_b, kn_w, kn_b):
    S = hidden_states.shape[1]
    hT = np.ascontiguousarray(
        np.asarray(hidden_states, np.float32)[0].T.astype(np.float16))
    pos = np.asarray(position_ids, np.float32)[0]  # [S]
    inv = 1.0 / (_ROPE_THETA ** (np.arange(0, _D, 2, dtype=np.float32) / _D))
    fr = pos[:, None] * inv[None, :]  # [S, D/2]
    emb = np.concatenate([fr, fr], axis=1)  # [S, D]
    cos = np.cos(emb).astype(np.float32)
    sin = np.sin(emb).astype(np.float32)

    half = _D // 2
    perm = np.concatenate([np.arange(half, _D), np.arange(0, half)])
    sign = np.concatenate([-np.ones(half, np.float32), np.ones(half, np.float32)])

    def coeffs(g, b):
        g = np.asarray(g, np.float32).reshape(_D)
        b = np.asarray(b, np.float32).reshape(_D)
        c1 = g[None, :] * cos  # [S, D]
        dd = (sign * g[perm])[None, :] * sin
        ee = b[None, :] * cos + (sign * b[perm])[None, :] * sin
        wide = lambda a: np.tile(a, (1, _HPC))  # [S, 512]
        # packed [S, 3*512]: c1 | dd | ee
        return np.concatenate(
            [wide(c1), wide(dd), wide(ee)], axis=1).astype(np.float16)

    ropeq = coeffs(qn_w, qn_b)
    ropek = coeffs(kn_w, kn_b)

    masks = np.zeros((4, 128, 512), np.float16)
    for t in range(4):
        kk = np.arange(128)[:, None] + t * 128
        qq = np.arange(512)[None, :]
        masks[t] = (kk <= qq).astype(np.float16)

    common = {"hT": hT, "ropeq": ropeq, "ropek": ropek, "masks": masks}
    Wq = np.asarray(Wq, np.float32).astype(np.float16)
    Wk = np.asarray(Wk, np.float32).astype(np.float16)
    Wv = np.asarray(Wv, np.float32).astype(np.float16)
    Wo16 = np.asarray(Wo, np.float32).astype(np.float16)
    in_maps = []
    for c in range(_NC):
        sl = slice(c * _CPW, (c + 1) * _CPW)
        m = dict(common)
        m["wq"] = np.ascontiguousarray(Wq[:, sl])
        m["wk"] = np.ascontiguousarray(Wk[:, sl])
        m["wv"] = np.ascontiguousarray(Wv[:, sl])
        m["wo"] = np.ascontiguousarray(Wo16[sl, :])
        in_maps.append(m)
    return in_maps


def kernel(**inputs) -> np.ndarray:
    from concourse.bass_utils import run_bass_kernel_spmd

    hidden_states = np.asarray(inputs["hidden_states"])
    S = hidden_states.shape[1]
    if S not in _cache:
        _cache[S] = _build(S)
    nc = _cache[S]

    in_maps = _host_prep(
        hidden_states,
        inputs["position_ids"],
        inputs["Wq"], inputs["Wk"], inputs["Wv"], inputs["Wo"],
        inputs["qn_w"], inputs["qn_b"], inputs["kn_w"], inputs["kn_b"],
    )
    res = run_bass_kernel_spmd(nc, in_maps, list(range(_NC)))
    out = np.zeros((S, _HID), np.float32)
    for c in range(_NC):
        out += res.results[c]["out"].astype(np.float32)
    return out.reshape(1, S, _HID)
